# revision 1
# baseline (speedup 1.0000x reference)
"""MGNN (gnn_message_passing) Trainium2 kernel.

Strategy (8 NeuronCores, destination-sharded, no collectives):
  - Each core owns N/8 = 6250 destination nodes. Host partitions the edge
    lists by destination row, sorts by (local) destination, and pads edge
    chunks so all 8 cores run an identical SPMD program.
  - Aggregation identity: agg_i = segsum(val * (h @ W_i^T)[col])
                                = segsum(val * h[col]) @ W_i^T
    so the gather table is h itself for all 3 metapaths; the per-metapath
    weight matmul is applied after aggregation (on [D, n] tiles).
  - On device (feature-major layout [D=128 partitions, nodes on free dim]):
      * batched indirect-DMA gathers of h rows (128 rows/chunk, 32 chunks
        per DMA instruction); out-of-range pad indices are skipped via
        bounds_check (no HBM traffic for padding).
      * segment-sum via one-hot matmul: S[e, d] = val_e * (iota[d]==doff_e),
        PSUM accumulation per 32-destination window.
      * FiLM folded into weights: gamma ∈ {g0, g1} by node_type, handled by
        sorting each core's nodes by type (host) and using two pre-scaled
        weight matrices W0 = diag(g0) W, W1 = diag(g1) W. seq_fts residual
        is matmul-accumulated into the same PSUM tile.
      * PReLU(u) = max(u, a*u) via two scalar-engine affine ops + vector max.
      * Semantics attention: tanh/score matmuls in feature-major form,
        softmax computed node-major after an SBUF reshape DMA.
  - Output is written feature-major [128, NCOL]; host transposes, strips
    padding, undoes the type-sort permutation and concatenates shards.
"""

import math
import os

import numpy as np


def _ensure_path():
    try:
        import concourse  # noqa: F401
    except ImportError:
        import sys

        for p in ("/opt/trn_rl_repo", "/root/.axon_site/_ro/trn_rl_repo"):
            if os.path.isdir(p) and p not in sys.path:
                sys.path.insert(0, p)


# ---------------------------------------------------------------------------
# configuration
# ---------------------------------------------------------------------------

N_CORES = 8
D = 128          # hidden dim (= partition count)
CHUNK = 128      # edges per matmul chunk (contraction dim)
WIN = 64         # destinations per one-hot window (S width)
KG = 16          # chunks per dma_gather instruction
BANK = 512      # psum bank width (f32 elems) = 16 windows
PAD_COL = 1 << 28  # out-of-bounds gather index for pad edges (skipped)

F32 = np.float32
I32 = np.int32


# ---------------------------------------------------------------------------
# host-side planning
# ---------------------------------------------------------------------------

def _round_up(x, m):
    return (x + m - 1) // m * m


def _plan(h, edge_rows, edge_cols, edge_vals, node_type):
    """Shard by destination, type-sort each shard, build padded chunk plan.

    Chunks are segregated by source half (dma_gather indices are int16, so
    the gather table is split at NLO = N//2). Returns (cfg, per_core).
    """
    N = h.shape[0]
    P = edge_rows.shape[0]
    npc = N // N_CORES
    assert npc * N_CORES == N
    NLO = N // 2
    assert NLO <= 32768 and (N - NLO) <= 32768

    # --- per-core destination shards, sorted by node_type (stable) ---
    shards = []
    for c in range(N_CORES):
        own = slice(c * npc, (c + 1) * npc)
        t = node_type[own]
        perm = np.argsort(t, kind="stable")  # sorted-rank -> original local id
        n0 = int((t == 0).sum())
        shards.append({"perm": perm, "n0": n0})

    max_n0 = max(s["n0"] for s in shards)
    max_n1 = max(npc - s["n0"] for s in shards)
    B0 = _round_up(max(max_n0, 1), BANK)
    NCOL = B0 + _round_up(max(max_n1, 1), BANK)
    NBANK = NCOL // BANK
    NWIN = NCOL // WIN

    # padded-column map per core: local node id -> column
    for s in shards:
        inv = np.empty(npc, dtype=np.int64)
        inv[s["perm"]] = np.arange(npc)  # original local id -> sorted rank
        col = np.where(inv < s["n0"], inv, B0 + (inv - s["n0"]))
        s["colmap"] = col

    # --- edge bucketing by (core, metapath, half, window) ---
    edge_data = [[None] * P for _ in range(N_CORES)]
    hist = np.zeros((2, N_CORES, P, NWIN), dtype=np.int64)
    for c in range(N_CORES):
        base = c * npc
        for m in range(P):
            er = edge_rows[m]
            mask = (er >= base) & (er < base + npc)
            dl = shards[c]["colmap"][er[mask] - base]
            cs = edge_cols[m][mask].astype(np.int64)
            vs = edge_vals[m][mask].astype(F32)
            half = (cs >= NLO).astype(np.int64)
            # sort by (window, half) so each (w, half) group is contiguous
            key = (dl // WIN) * 2 + half
            order = np.argsort(key, kind="stable")
            dl = dl[order]
            cs = cs[order]
            vs = vs[order]
            half = half[order]
            w = dl // WIN
            for hf in range(2):
                hist[hf, c, m] += np.bincount(w[half == hf], minlength=NWIN)
            edge_data[c][m] = (dl, cs, vs, w, half)

    cl = np.maximum(1, -(-hist[0].max(axis=0) // CHUNK))   # [P, NWIN] lo
    ch = -(-hist[1].max(axis=0) // CHUNK)                  # [P, NWIN] hi
    counts2 = np.stack([cl, ch])                           # [2, P, NWIN]
    nch = [int(cl.sum()), int(ch.sum())]
    nch_pad = [_round_up(x, KG) for x in nch]

    # chunk slot base per (half, m, w) within its stream
    base_slot = np.zeros((2, P, NWIN), dtype=np.int64)
    for hf in range(2):
        flat = counts2[hf].reshape(-1)
        base_slot[hf].reshape(-1)[1:] = np.cumsum(flat)[:-1]

    per_core = []
    for c in range(N_CORES):
        streams = []
        for hf in range(2):
            nitems = nch_pad[hf] * CHUNK
            streams.append({
                "idx": np.full(nitems, -1, dtype=np.int64),
                "doff": np.zeros(nitems, dtype=F32),
                "val": np.zeros(nitems, dtype=F32),
            })
            # in-plan chunks: pad items default to row 0 / val 0
            ninplan = int(counts2[hf].sum()) * CHUNK
            streams[hf]["idx"][:ninplan] = 0
        for m in range(P):
            dl, cs, vs, w, half = edge_data[c][m]
            for hf in range(2):
                sel = half == hf
                wm_ = w[sel]
                starts = np.searchsorted(wm_, np.arange(NWIN))
                rank = np.arange(len(wm_)) - starts[wm_]
                slot = base_slot[hf, m, wm_] * CHUNK + rank
                st = streams[hf]
                st["idx"][slot] = cs[sel] - hf * NLO
                st["doff"][slot] = (dl[sel] - wm_ * WIN).astype(F32)
                st["val"][slot] = vs[sel]
        pc = {"perm": shards[c]["perm"], "n0": shards[c]["n0"]}
        for hf, tag in ((0, "L"), (1, "H")):
            st = streams[hf]
            # idx items wrapped in 16 partitions, replicated to 128
            iw = np.ascontiguousarray(
                st["idx"].reshape(-1, 16).T).astype(np.int16)   # [16, items/16]
            pc["idx" + tag] = np.tile(iw, (8, 1))               # [128, items/16]
            pc["doff" + tag] = np.ascontiguousarray(
                st["doff"].reshape(-1, CHUNK).T)                # [128, nch_pad]
            pc["val" + tag] = np.ascontiguousarray(
                st["val"].reshape(-1, CHUNK).T)
        per_core.append(pc)

    # per-gather-batch valid index counts (same for all cores by construction)
    nreg = []
    for hf in range(2):
        ninplan = int(counts2[hf].sum())
        nb = nch_pad[hf] // KG
        r = []
        for g in range(nb):
            lo_c = g * KG
            r.append(max(0, min(ninplan - lo_c, KG)) * CHUNK)
        nreg.append(r)

    cfg = {
        "N": N,
        "NLO": NLO,
        "P": P,
        "npc": npc,
        "B0": B0,
        "NCOL": NCOL,
        "NBANK": NBANK,
        "NWIN": NWIN,
        "counts2": counts2,
        "nch_pad": nch_pad,
        "nreg": nreg,
    }
    return cfg, per_core


def _pack_weights(cfg, W_fc, prelu_a, Wg, bg, Wb, bb, film_bias,
                  att_W1, att_b1, att_w2):
    """Pack small weights into two dense blobs (replicated to every core)."""
    P = cfg["P"]
    # wmats: per meta [W0T, W1T, WfcT], then att_W1T  -> [128, (3P+1)*128]
    blocks = []
    for m in range(P):
        g0 = (Wg[m][:, 0] + bg[m]).astype(F32)  # [D]
        g1 = (Wg[m][:, 1] + bg[m]).astype(F32)
        WT = W_fc[m].T.astype(F32)              # [fi, fo]
        blocks += [WT * g0[None, :], WT * g1[None, :], WT]
    blocks.append(att_W1.T.astype(F32))          # lhsT[d, hid]
    wmats = np.ascontiguousarray(np.concatenate(blocks, axis=1))

    # cvec [128, WIN+16]: iota window, b1, w2,
    # per-meta (bfb0, bfb1, a*bfb0, a*bfb1)
    cvec = np.zeros((D, WIN + 16), dtype=F32)
    cvec[:, :WIN] = np.arange(WIN, dtype=F32)[None, :]
    cvec[:, WIN] = att_b1.astype(F32)
    cvec[:, WIN + 1] = att_w2.astype(F32)
    for m in range(P):
        a = float(prelu_a[m])
        bfb0 = (Wb[m][:, 0] + bb[m] + film_bias[m]).astype(F32)
        bfb1 = (Wb[m][:, 1] + bb[m] + film_bias[m]).astype(F32)
        cvec[:, WIN + 2 + 4 * m] = bfb0
        cvec[:, WIN + 3 + 4 * m] = bfb1
        cvec[:, WIN + 4 + 4 * m] = a * bfb0
        cvec[:, WIN + 5 + 4 * m] = a * bfb1
    return wmats, cvec


# ---------------------------------------------------------------------------
# device program
# ---------------------------------------------------------------------------

def _build_program(cfg, alphas, stage=99):
    _ensure_path()
    import concourse.bass as bass  # noqa: F401
    import concourse.tile as tile
    from concourse import bacc, mybir

    P = cfg["P"]
    NCOL = cfg["NCOL"]
    NBANK = cfg["NBANK"]
    counts2 = cfg["counts2"]
    nch_pad = cfg["nch_pad"]
    nreg = cfg["nreg"]
    N = cfg["N"]
    NLO = cfg["NLO"]
    dt = mybir.dt
    f32 = dt.float32

    nc = bacc.Bacc(
        "TRN2",
        target_bir_lowering=False,
        debug=False,
        enable_asserts=False,
        num_devices=N_CORES,
    )

    h_tab = nc.dram_tensor("h_tab", [N, D], f32, kind="ExternalInput").ap()
    hT = nc.dram_tensor("hT", [D, NCOL], f32, kind="ExternalInput").ap()
    idxd = {}
    doffd = {}
    vald = {}
    for hf, tag in ((0, "L"), (1, "H")):
        ni = max(nch_pad[hf] * CHUNK // 16, 1)
        idxd[hf] = nc.dram_tensor(f"idx{tag}", [CHUNK, ni], dt.int16,
                                  kind="ExternalInput").ap()
        nch1 = max(nch_pad[hf], 1)
        doffd[hf] = nc.dram_tensor(f"doff{tag}", [CHUNK, nch1], f32,
                                   kind="ExternalInput").ap()
        vald[hf] = nc.dram_tensor(f"val{tag}", [CHUNK, nch1], f32,
                                  kind="ExternalInput").ap()
    wmatsd = nc.dram_tensor("wmats", [D, (3 * P + 1) * D], f32,
                            kind="ExternalInput").ap()
    cvecd = nc.dram_tensor("cvec", [D, WIN + 16], f32, kind="ExternalInput").ap()
    outd = nc.dram_tensor("outT", [D, NCOL], f32, kind="ExternalOutput").ap()
    zspill = nc.dram_tensor("z_spill", [P, D, NCOL], f32, kind="Internal").ap()

    half_tab = {0: h_tab[0:NLO, :], 1: h_tab[NLO:N, :]}

    with tile.TileContext(nc) as tc, tc.tile_pool(name="const", bufs=1) as cpool, \
            tc.tile_pool(name="gpool", bufs=2) as gpool, \
            tc.tile_pool(name="spool", bufs=2) as spool, \
            tc.tile_pool(name="mpool", bufs=2) as mpool, \
            tc.tile_pool(name="work", bufs=2) as work, \
            tc.tile_pool(name="ps_agg", bufs=3, space="PSUM") as ps_agg, \
            tc.tile_pool(name="ps_misc", bufs=2, space="PSUM") as ps_misc, \
            tc.tile_pool(name="ps_attn", bufs=2, space="PSUM") as ps_attn:

        # ---- constants / resident inputs ----
        hT_t = cpool.tile([D, NCOL], f32, tag="hT", name="hT")
        nc.sync.dma_start(out=hT_t[:], in_=hT)
        wm_t = cpool.tile([D, (3 * P + 1) * D], f32, tag="wm", name="wm")
        nc.sync.dma_start(out=wm_t[:], in_=wmatsd)
        cv_t = cpool.tile([D, WIN + 16], f32, tag="cv", name="cv")
        nc.sync.dma_start(out=cv_t[:], in_=cvecd)
        ones_t = cpool.tile([65, D], f32, tag="ones", name="ones")
        nc.vector.memset(ones_t[:], 1.0)

        def wmat(i):  # [128,128] lhsT block i
            return wm_t[:, i * D:(i + 1) * D]

        attW1T = wmat(3 * P)
        iota = cv_t[:, 0:WIN]
        b1c = cv_t[:, WIN:WIN + 1]
        w2c = cv_t[:, WIN + 1:WIN + 2]

        # partitions 0/32/64 hold s_m then beta_m (WAR-serialized)
        rows_t = cpool.tile([65, NCOL], f32, tag="rows", name="rows")

        # ---- gather + S build, two half streams ----
        # Pre-zero gather-pool slots: trailing pad indices (-1) are skipped
        # by dma_gather (no write); uninitialized SBUF may hold NaN which
        # S=0 would not mask (NaN*0=NaN in the matmul).
        for tg in ("gL", "gH"):
            for _ in range(2):
                gw = gpool.tile([CHUNK, KG * D], f32, tag=tg, name=tg)
                nc.vector.memset(gw[:], 0.0)

        gtiles = {}
        stiles = {}

        def ensure_batch(hf, g):
            if (hf, g) in gtiles:
                return
            tg = "gL" if hf == 0 else "gH"
            ix = mpool.tile([CHUNK, KG * CHUNK // 16], dt.int16,
                            tag="ix" + tg, name="ix" + tg)
            nc.sync.dma_start(
                out=ix[:],
                in_=idxd[hf][:, g * (KG * CHUNK // 16):(g + 1) * (KG * CHUNK // 16)])
            mdoff = mpool.tile([CHUNK, KG], f32, tag="md" + tg, name="md" + tg)
            nc.sync.dma_start(out=mdoff[:], in_=doffd[hf][:, g * KG:(g + 1) * KG])
            mval = mpool.tile([CHUNK, KG], f32, tag="mv" + tg, name="mv" + tg)
            nc.sync.dma_start(out=mval[:], in_=vald[hf][:, g * KG:(g + 1) * KG])
            gt = gpool.tile([CHUNK, KG * D], f32, tag=tg, name=tg)
            nc.gpsimd.dma_gather(
                out_ap=gt[:].rearrange("p (k d) -> p k d", k=KG),
                in_ap=half_tab[hf],
                idxs_ap=ix[:],
                num_idxs=KG * CHUNK,
                num_idxs_reg=int(nreg[hf][g]),
                elem_size=D,
                single_packet=False,
            )
            eq = spool.tile([CHUNK, KG * WIN], f32, tag="eq", name="eq",
                            bufs=1)
            st = spool.tile([CHUNK, KG * WIN], f32, tag="s" + tg,
                            name="s" + tg)
            nc.vector.tensor_tensor(
                out=eq[:],
                in0=iota.unsqueeze(1).to_broadcast([CHUNK, KG, WIN]),
                in1=mdoff[:].unsqueeze(2).to_broadcast([CHUNK, KG, WIN]),
                op=mybir.AluOpType.is_equal,
            )
            nc.vector.tensor_tensor(
                out=st[:],
                in0=eq[:],
                in1=mval[:].unsqueeze(2).to_broadcast([CHUNK, KG, WIN]),
                op=mybir.AluOpType.mult,
            )
            gtiles[(hf, g)] = gt
            stiles[(hf, g)] = st

        chunk_c = [0, 0]
        WPB = BANK // WIN  # windows per bank

        for m in range(P):
            for b in range(NBANK):
                agg = ps_agg.tile([D, BANK], f32, space="PSUM", tag="agg", name="agg")
                for wl in range(WPB):
                    w = b * WPB + wl
                    tot = int(counts2[0, m, w]) + int(counts2[1, m, w])
                    j = 0
                    for hf in range(2):
                        for _ in range(int(counts2[hf, m, w])):
                            g, cl = divmod(chunk_c[hf], KG)
                            ensure_batch(hf, g)
                            nc.tensor.matmul(
                                out=agg[:, wl * WIN:(wl + 1) * WIN],
                                lhsT=gtiles[(hf, g)][:, cl * D:(cl + 1) * D],
                                rhs=stiles[(hf, g)][:, cl * WIN:(cl + 1) * WIN],
                                start=(j == 0),
                                stop=(j == tot - 1),
                            )
                            chunk_c[hf] += 1
                            j += 1
                # evacuate A@h bank to SBUF (scalar engine copy)
                aggh = work.tile([D, BANK], f32, tag="aggh", name="aggh")
                nc.scalar.copy(out=aggh[:], in_=agg[:])
                if stage < 2:
                    nc.sync.dma_start(out=outd[:, slice(b * BANK, (b + 1) * BANK)],
                                      in_=aggh[:])
                    continue
                # z_pre^T = W_t . aggh + W . hT   (accumulated in PSUM)
                fps = ps_misc.tile([D, BANK], f32, space="PSUM", tag="fps", name="fps")
                wsel = 3 * m + (0 if b < cfg["B0"] // BANK else 1)
                csl = slice(b * BANK, (b + 1) * BANK)
                nc.tensor.matmul(out=fps[:], lhsT=wmat(wsel), rhs=aggh[:],
                                 start=True, stop=False)
                nc.tensor.matmul(out=fps[:], lhsT=wmat(3 * m + 2),
                                 rhs=hT_t[:, csl], start=False, stop=True)
                # PReLU(u + bfb) = max(u + bfb, a*u + a*bfb)
                ty = 0 if b < cfg["B0"] // BANK else 1
                bfb = cv_t[:, WIN + 2 + 4 * m + ty:WIN + 3 + 4 * m + ty]
                abfb = cv_t[:, WIN + 4 + 4 * m + ty:WIN + 5 + 4 * m + ty]
                t0 = work.tile([D, BANK], f32, tag="t0", name="t0")
                t1 = work.tile([D, BANK], f32, tag="t1", name="t1")
                nc.scalar.activation(t0[:], fps[:],
                                     mybir.ActivationFunctionType.Identity,
                                     bias=bfb, scale=1.0)
                nc.scalar.activation(t1[:], fps[:],
                                     mybir.ActivationFunctionType.Identity,
                                     bias=abfb, scale=float(alphas[m]))
                zb = work.tile([D, BANK], f32, tag="zb", name="zb")
                nc.vector.tensor_tensor(out=zb[:], in0=t0[:],
                                        in1=t1[:], op=mybir.AluOpType.max)
                nc.sync.dma_start(out=zspill[m, :, csl], in_=zb[:])
                # attention scores for this bank
                if stage < 3:
                    continue
                aps = ps_attn.tile([D, BANK], f32, space="PSUM", tag="at", name="at")
                nc.tensor.matmul(out=aps[:], lhsT=attW1T, rhs=zb[:],
                                 start=True, stop=True)
                th = work.tile([D, BANK], f32, tag="tanh", name="tanh")
                nc.scalar.activation(th[:], aps[:],
                                     mybir.ActivationFunctionType.Tanh,
                                     bias=b1c, scale=1.0)
                sps = ps_attn.tile([1, BANK], f32, space="PSUM", tag="at", name="at")
                nc.tensor.matmul(out=sps[:], lhsT=w2c, rhs=th[:],
                                 start=True, stop=True)
                nc.scalar.copy(out=rows_t[32 * m:32 * m + 1, csl], in_=sps[:])

        assert chunk_c[0] == int(counts2[0].sum())
        assert chunk_c[1] == int(counts2[1].sum())

        # ---- softmax over metapaths (node-major [128, NCOL/128]) ----
        if stage < 4:
            if stage >= 2:
                for b in range(NBANK):
                    csl = slice(b * BANK, (b + 1) * BANK)
                    zl0 = work.tile([D, BANK], f32, tag="zl", name="zl", bufs=4)
                    nc.sync.dma_start(out=zl0[:], in_=zspill[0, :, csl])
                    nc.sync.dma_start(out=outd[:, csl], in_=zl0[:])
        if stage >= 4:
            NMW = NCOL // D
            s_nm = [work.tile([D, NMW], f32, tag=f"snm{m}", name=f"snm{m}",
                              bufs=1) for m in range(P)]
            for m in range(P):
                nc.sync.dma_start(out=s_nm[m][:], in_=rows_t[32 * m:32 * m + 1, :])
            mx = work.tile([D, NMW], f32, tag="mx", name="mx")
            nc.vector.tensor_tensor(out=mx[:], in0=s_nm[0][:], in1=s_nm[1][:],
                                    op=mybir.AluOpType.max)
            nc.vector.tensor_tensor(out=mx[:], in0=mx[:], in1=s_nm[2][:],
                                    op=mybir.AluOpType.max)
            ex = [work.tile([D, NMW], f32, tag=f"ex{m}", name=f"ex{m}", bufs=1)
                  for m in range(P)]
            for m in range(P):
                d = work.tile([D, NMW], f32, tag="sd", name="sd")
                nc.vector.tensor_tensor(out=d[:], in0=s_nm[m][:], in1=mx[:],
                                        op=mybir.AluOpType.subtract)
                nc.scalar.activation(ex[m][:], d[:],
                                     mybir.ActivationFunctionType.Exp)
            sm = work.tile([D, NMW], f32, tag="sm", name="sm")
            nc.vector.tensor_tensor(out=sm[:], in0=ex[0][:], in1=ex[1][:],
                                    op=mybir.AluOpType.add)
            nc.vector.tensor_tensor(out=sm[:], in0=sm[:], in1=ex[2][:],
                                    op=mybir.AluOpType.add)
            rc = work.tile([D, NMW], f32, tag="rc", name="rc")
            nc.vector.reciprocal(out=rc[:], in_=sm[:])
            for m in range(P):
                bt = work.tile([D, NMW], f32, tag="bt", name="bt")
                nc.vector.tensor_tensor(out=bt[:], in0=ex[m][:], in1=rc[:],
                                        op=mybir.AluOpType.mult)
                nc.sync.dma_start(out=rows_t[32 * m:32 * m + 1, :], in_=bt[:])

            # ---- final combine per bank: out = sum_m beta_m * z_m + hT ----
            for b in range(NBANK):
                csl = slice(b * BANK, (b + 1) * BANK)
                acc = work.tile([D, BANK], f32, tag="acc", name="acc")
                tmp = work.tile([D, BANK], f32, tag="tmp", name="tmp")
                for m in range(P):
                    zl = work.tile([D, BANK], f32, tag="zl", name="zl", bufs=4)
                    nc.sync.dma_start(out=zl[:], in_=zspill[m, :, csl])
                    bps = ps_misc.tile([D, BANK], f32, space="PSUM", tag="fps", name="fps")
                    nc.tensor.matmul(out=bps[:], lhsT=ones_t[32 * m:32 * m + 1, :],
                                     rhs=rows_t[32 * m:32 * m + 1, csl],
                                     start=True, stop=True)
                    dst = acc if m == 0 else tmp
                    nc.vector.tensor_tensor(out=dst[:], in0=zl[:],
                                            in1=bps[:], op=mybir.AluOpType.mult)
                    if m > 0:
                        nc.vector.tensor_tensor(out=acc[:], in0=acc[:],
                                                in1=tmp[:],
                                                op=mybir.AluOpType.add)
                nc.vector.tensor_tensor(out=acc[:], in0=acc[:], in1=hT_t[:, csl],
                                        op=mybir.AluOpType.add)
                nc.sync.dma_start(out=outd[:, csl], in_=acc[:])

    nc.compile()
    return nc


# ---------------------------------------------------------------------------
# entry point
# ---------------------------------------------------------------------------

def kernel(h, edge_rows, edge_cols, edge_vals, node_type,
           W_fc, prelu_a, Wg, bg, Wb, bb, film_bias,
           att_W1, att_b1, att_w2, _run_opts=None):
    _ensure_path()
    from concourse import bass_utils

    h = np.asarray(h, dtype=F32)
    edge_rows = np.asarray(edge_rows)
    edge_cols = np.asarray(edge_cols)
    edge_vals = np.asarray(edge_vals, dtype=F32)
    node_type = np.asarray(node_type)

    cfg, per_core = _plan(h, edge_rows, edge_cols, edge_vals, node_type)
    wmats, cvec = _pack_weights(cfg, np.asarray(W_fc), np.asarray(prelu_a),
                                np.asarray(Wg), np.asarray(bg),
                                np.asarray(Wb), np.asarray(bb),
                                np.asarray(film_bias), np.asarray(att_W1),
                                np.asarray(att_b1), np.asarray(att_w2))

    nc = _build_program(cfg, np.asarray(prelu_a, dtype=F32))

    npc = cfg["npc"]
    B0 = cfg["B0"]
    NCOL = cfg["NCOL"]
    in_maps = []
    for c in range(N_CORES):
        pc = per_core[c]
        hT_own = np.zeros((D, NCOL), dtype=F32)
        own = h[c * npc:(c + 1) * npc]       # [npc, D]
        srt = own[pc["perm"]]                 # type-sorted rows
        n0 = pc["n0"]
        hT_own[:, :n0] = srt[:n0].T
        hT_own[:, B0:B0 + (npc - n0)] = srt[n0:].T
        im = {
            "h_tab": h,
            "hT": hT_own,
            "wmats": wmats,
            "cvec": cvec,
        }
        for tag in ("L", "H"):
            for nm in ("idx", "doff", "val"):
                arr = pc[nm + tag]
                if arr.shape[1] == 0:  # empty stream: dram tensor padded to 1
                    arr = np.zeros(
                        (CHUNK, 1),
                        dtype=np.int16 if nm == "idx" else F32)
                    if nm == "idx":
                        arr -= 1
                im[nm + tag] = arr
        in_maps.append(im)

    run_kwargs = dict(_run_opts or {})
    res = bass_utils.run_bass_kernel_spmd(
        nc, in_maps, core_ids=list(range(N_CORES)), **run_kwargs
    )

    out = np.empty((cfg["N"], D), dtype=F32)
    for c in range(N_CORES):
        pc = per_core[c]
        n0 = pc["n0"]
        zT = res.results[c]["outT"]           # [D, NCOL]
        real = np.concatenate(
            [zT[:, :n0], zT[:, B0:B0 + (npc - n0)]], axis=1
        ).T                                    # [npc, D] sorted order
        shard = np.empty((npc, D), dtype=F32)
        shard[pc["perm"]] = real
        out[c * npc:(c + 1) * npc] = shard
    if isinstance(_run_opts, dict):
        _run_opts["_result"] = res
    return out



# revision 21
# speedup vs baseline: 1.0245x; 1.0245x over previous
"""MGNN (gnn_message_passing) Trainium2 kernel — v2.

Strategy (8 NeuronCores, destination-sharded, no collectives):
  - Each core owns N/8 = 6250 destination nodes; host partitions edge lists
    by destination row, type-sorts destinations, pads per-(metapath, window)
    chunk groups so all 8 cores run one SPMD program (as v1).
  - NEW: the full node table h is kept RESIDENT IN SBUF as bf16 (12.8 MB,
    token-striped layout), loaded once sequentially. Per-edge source rows are
    then gathered SBUF->SBUF via transpose-mode dma_gather — removing the
    ~120 MB/core of latency-bound random HBM reads that dominated v1.
  - The transpose-gather output is feature-major [D, edges]; the mandatory
    re-transpose to edge-major is fused with the FiLM weight multiply:
        T[e, j] = sum_d G[d, e] * (diag(gamma_ty) W_m)^T[d, j]
    one matmul per 128-edge chunk (PSUM), evacuated to SBUF in 512-col
    batches alternating scalar/vector engines.
  - Segment-sum via one-hot matmul per 64-dest window (S built on DVE in
    bf16), accumulating z_pre^T directly in PSUM; the seq_fts residual
    (W_m h^T) is matmul-accumulated into the same PSUM bank.
  - Loop order is bank-major: per bank all 3 metapath z's stay in SBUF
    (bf16), softmax over metapaths is computed in-place via PE broadcast
    tricks (no z spill to HBM, no node-major reshape DMAs).
  - All matmul operands are bf16 (PSUM accumulation f32).
"""

import math
import os

import numpy as np


def _ensure_path():
    try:
        import concourse  # noqa: F401
    except ImportError:
        import sys

        for p in ("/opt/trn_rl_repo", "/root/.axon_site/_ro/trn_rl_repo"):
            if os.path.isdir(p) and p not in sys.path:
                sys.path.insert(0, p)


# ---------------------------------------------------------------------------
# configuration
# ---------------------------------------------------------------------------

N_CORES = 8
D = 128          # hidden dim (= partition count)
CHUNK = 128      # edges per matmul chunk (contraction dim)
WIN = 64         # destinations per one-hot window (S width)
KG = 16          # chunks per dma_gather instruction
BANK = 512       # psum bank width (f32 elems) = 8 windows
WPB = BANK // WIN
NLO = 32768      # token split for int16 gather indices (256 ranks)

F32 = np.float32
I32 = np.int32


def _round_up(x, m):
    return (x + m - 1) // m * m


def _to_bf16_bits(x):
    x = np.ascontiguousarray(x, dtype=np.float32)
    return ((x.view(np.uint32) + 0x8000) >> 16).astype(np.uint16)


# ---------------------------------------------------------------------------
# host-side planning
# ---------------------------------------------------------------------------

def _plan(h, edge_rows, edge_cols, edge_vals, node_type):
    """Shard by destination, type-sort each shard, build padded chunk plan.

    Chunk groups are keyed (half, metapath, window) and laid out in the two
    half streams in CONSUMPTION order: bank-major, then metapath, then
    window, then half. Returns (cfg, per_core).
    """
    N = h.shape[0]
    P = edge_rows.shape[0]
    npc = N // N_CORES
    assert npc * N_CORES == N
    assert NLO <= 32768 and (N - NLO) <= 32767

    # --- per-core destination shards, sorted by node_type (stable) ---
    shards = []
    for c in range(N_CORES):
        own = slice(c * npc, (c + 1) * npc)
        t = node_type[own]
        perm = np.argsort(t, kind="stable")  # sorted-rank -> original local id
        n0 = int((t == 0).sum())
        shards.append({"perm": perm, "n0": n0})

    max_n0 = max(s["n0"] for s in shards)
    max_n1 = max(npc - s["n0"] for s in shards)
    B0 = _round_up(max(max_n0, 1), BANK)
    NCOL = B0 + _round_up(max(max_n1, 1), BANK)
    NBANK = NCOL // BANK
    NWIN = NCOL // WIN

    # padded-column map per core: local node id -> column
    for s in shards:
        inv = np.empty(npc, dtype=np.int64)
        inv[s["perm"]] = np.arange(npc)  # original local id -> sorted rank
        col = np.where(inv < s["n0"], inv, B0 + (inv - s["n0"]))
        s["colmap"] = col

    # --- edge bucketing by (core, metapath, half, window) ---
    edge_data = [[None] * P for _ in range(N_CORES)]
    hist = np.zeros((2, N_CORES, P, NWIN), dtype=np.int64)
    for c in range(N_CORES):
        base = c * npc
        for m in range(P):
            er = edge_rows[m]
            mask = (er >= base) & (er < base + npc)
            dl = shards[c]["colmap"][er[mask] - base]
            cs = edge_cols[m][mask].astype(np.int64)
            vs = edge_vals[m][mask].astype(F32)
            half = (cs >= NLO).astype(np.int64)
            # sort by (window, half) so each (w, half) group is contiguous
            key = (dl // WIN) * 2 + half
            order = np.argsort(key, kind="stable")
            dl = dl[order]
            cs = cs[order]
            vs = vs[order]
            half = half[order]
            w = dl // WIN
            for hf in range(2):
                hist[hf, c, m] += np.bincount(w[half == hf], minlength=NWIN)
            edge_data[c][m] = (dl, cs, vs, w, half)

    cl = np.maximum(1, -(-hist[0].max(axis=0) // CHUNK))   # [P, NWIN] lo
    ch = -(-hist[1].max(axis=0) // CHUNK)                  # [P, NWIN] hi
    counts2 = np.stack([cl, ch])                           # [2, P, NWIN]
    nch = [int(cl.sum()), int(ch.sum())]
    nch_pad = [_round_up(max(x, 1), KG) for x in nch]

    # chunk slot base per (half, m, w), stream order = (bank, m, wl)
    base_slot = np.zeros((2, P, NWIN), dtype=np.int64)
    for hf in range(2):
        acc = 0
        for b in range(NBANK):
            for m in range(P):
                for wl in range(WPB):
                    w = b * WPB + wl
                    base_slot[hf, m, w] = acc
                    acc += int(counts2[hf, m, w])
        assert acc == nch[hf]

    per_core = []
    for c in range(N_CORES):
        streams = []
        for hf in range(2):
            nitems = nch_pad[hf] * CHUNK
            streams.append({
                "idx": np.full(nitems, -1, dtype=np.int64),
                "doff": np.zeros(nitems, dtype=F32),
                "val": np.zeros(nitems, dtype=F32),
            })
            # in-plan chunks: pad items default to row 0 / val 0
            ninplan = nch[hf] * CHUNK
            streams[hf]["idx"][:ninplan] = 0
        for m in range(P):
            dl, cs, vs, w, half = edge_data[c][m]
            for hf in range(2):
                sel = half == hf
                wm_ = w[sel]
                starts = np.searchsorted(wm_, np.arange(NWIN))
                rank = np.arange(len(wm_)) - starts[wm_]
                slot = base_slot[hf, m, wm_] * CHUNK + rank
                st = streams[hf]
                st["idx"][slot] = cs[sel] - hf * NLO
                st["doff"][slot] = (dl[sel] - wm_ * WIN).astype(F32)
                st["val"][slot] = vs[sel]
        pc = {"perm": shards[c]["perm"], "n0": shards[c]["n0"]}
        for hf, tag in ((0, "L"), (1, "H")):
            st = streams[hf]
            # idx items wrapped in 16 partitions, replicated to 128
            iw = np.ascontiguousarray(
                st["idx"].reshape(-1, 16).T).astype(np.int16)   # [16, items/16]
            pc["idx" + tag] = np.tile(iw, (8, 1))               # [128, items/16]
            pc["doff" + tag] = np.ascontiguousarray(
                st["doff"].reshape(-1, CHUNK).T)                # [128, nch_pad]
            pc["val" + tag] = np.ascontiguousarray(
                st["val"].reshape(-1, CHUNK).T)
        per_core.append(pc)

    # per-gather-batch valid index counts (same for all cores by construction)
    nreg = []
    for hf in range(2):
        ninplan = nch[hf]
        nb = nch_pad[hf] // KG
        r = []
        for g in range(nb):
            lo_c = g * KG
            r.append(max(0, min(ninplan - lo_c, KG)) * CHUNK)
        nreg.append(r)

    cfg = {
        "N": N,
        "P": P,
        "npc": npc,
        "B0": B0,
        "NCOL": NCOL,
        "NBANK": NBANK,
        "NWIN": NWIN,
        "counts2": counts2,
        "nch_pad": nch_pad,
        "nreg": nreg,
    }
    return cfg, per_core


def _pack_weights(cfg, W_fc, prelu_a, Wg, bg, Wb, bb, film_bias,
                  att_W1, att_b1, att_w2):
    """Pack small weights: bf16 matmul blob + f32 constant vector."""
    P = cfg["P"]
    # wmats (bf16): per meta [W0T, W1T, WfcT], then att_W1T, a w2-column
    # block, then a bf16 iota row  -> [128, (3P+2)*128 + WIN]
    blocks = []
    for m in range(P):
        g0 = (Wg[m][:, 0] + bg[m]).astype(F32)  # [D]
        g1 = (Wg[m][:, 1] + bg[m]).astype(F32)
        WT = W_fc[m].T.astype(F32)              # [fi, fo]
        blocks += [WT * g0[None, :], WT * g1[None, :], WT]
    blocks.append(att_W1.T.astype(F32))          # lhsT[d, hid]
    extra = np.zeros((D, D), dtype=F32)
    extra[:, 0] = att_w2.astype(F32)             # w2 column (lhsT [128,1])
    blocks.append(extra)
    blocks.append(np.tile(np.arange(WIN, dtype=F32), (D, 1)))  # iota16
    wmats = np.concatenate(blocks, axis=1)
    wmats16 = _to_bf16_bits(wmats)               # uint16 view

    # cvec f32 [128, WIN+16]: b1, per-meta (bfb0, bfb1, a*bfb0, a*bfb1)
    cvec = np.zeros((D, WIN + 16), dtype=F32)
    cvec[:, WIN] = att_b1.astype(F32)
    for m in range(P):
        a = float(prelu_a[m])
        bfb0 = (Wb[m][:, 0] + bb[m] + film_bias[m]).astype(F32)
        bfb1 = (Wb[m][:, 1] + bb[m] + film_bias[m]).astype(F32)
        cvec[:, WIN + 2 + 4 * m] = bfb0
        cvec[:, WIN + 3 + 4 * m] = bfb1
        cvec[:, WIN + 4 + 4 * m] = a * bfb0
        cvec[:, WIN + 5 + 4 * m] = a * bfb1
    return wmats16, cvec


# ---------------------------------------------------------------------------
# device program
# ---------------------------------------------------------------------------

def _build_program(cfg, alphas):
    _ensure_path()
    import concourse.bass as bass  # noqa: F401
    import concourse.tile as tile
    from concourse import bacc, mybir

    P = cfg["P"]
    NCOL = cfg["NCOL"]
    NBANK = cfg["NBANK"]
    B0 = cfg["B0"]
    counts2 = cfg["counts2"]
    nch_pad = cfg["nch_pad"]
    nreg = cfg["nreg"]
    N = cfg["N"]
    NRANK = _round_up(N, 128) // 128
    NTAB = NRANK * 128
    dt = mybir.dt
    f32 = dt.float32
    bf16 = dt.bfloat16

    nc = bacc.Bacc(
        "TRN2",
        target_bir_lowering=False,
        debug=False,
        enable_asserts=False,
        num_devices=N_CORES,
    )

    tabd = nc.dram_tensor("tab", [D, NTAB], bf16, kind="ExternalInput").ap()
    hTd = nc.dram_tensor("hT", [D, NCOL], bf16, kind="ExternalInput").ap()
    idxd = {}
    doffd = {}
    vald = {}
    for hf, tag in ((0, "L"), (1, "H")):
        ni = max(nch_pad[hf] * CHUNK // 16, 1)
        idxd[hf] = nc.dram_tensor(f"idx{tag}", [CHUNK, ni], dt.int16,
                                  kind="ExternalInput").ap()
        nch1 = max(nch_pad[hf], 1)
        doffd[hf] = nc.dram_tensor(f"doff{tag}", [CHUNK, nch1], bf16,
                                   kind="ExternalInput").ap()
        vald[hf] = nc.dram_tensor(f"val{tag}", [CHUNK, nch1], bf16,
                                  kind="ExternalInput").ap()
    wmatsd = nc.dram_tensor("wmats", [D, (3 * P + 2) * D + WIN], bf16,
                            kind="ExternalInput").ap()
    cvecd = nc.dram_tensor("cvec", [D, WIN + 16], f32, kind="ExternalInput").ap()
    outd = nc.dram_tensor("outT", [D, NCOL], f32, kind="ExternalOutput").ap()

    GBUFS = 3

    with tile.TileContext(nc) as tc, tc.tile_pool(name="const", bufs=1) as cpool, \
            tc.tile_pool(name="gpool", bufs=GBUFS) as gpool, \
            tc.tile_pool(name="spool", bufs=3) as spool, \
            tc.tile_pool(name="mpool", bufs=3) as mpool, \
            tc.tile_pool(name="tpool", bufs=3) as tpool, \
            tc.tile_pool(name="work", bufs=2) as work, \
            tc.tile_pool(name="ps_t", bufs=2, space="PSUM") as ps_t, \
            tc.tile_pool(name="ps_agg", bufs=2, space="PSUM") as ps_agg, \
            tc.tile_pool(name="ps_attn", bufs=2, space="PSUM") as ps_attn, \
            tc.tile_pool(name="ps_bc", bufs=2, space="PSUM") as ps_bc:

        # ---- constants / resident inputs ----
        tab_t = cpool.tile([D, NTAB], bf16, tag="tab", name="tab")
        nc.sync.dma_start(out=tab_t[:], in_=tabd)
        hT_t = cpool.tile([D, NCOL], bf16, tag="hT", name="hT")
        nc.sync.dma_start(out=hT_t[:], in_=hTd)
        wm_t = cpool.tile([D, (3 * P + 2) * D + WIN], bf16, tag="wm",
                          name="wm")
        nc.sync.dma_start(out=wm_t[:], in_=wmatsd)
        cv_t = cpool.tile([D, WIN + 16], f32, tag="cv", name="cv")
        nc.sync.dma_start(out=cv_t[:], in_=cvecd)

        def wmat(i):  # [128,128] bf16 lhsT block i
            return wm_t[:, i * D:(i + 1) * D]

        attW1T = wmat(3 * P)
        w2col = wm_t[:, (3 * P + 1) * D:(3 * P + 1) * D + 1]       # [128,1]
        iota = wm_t[:, (3 * P + 2) * D:(3 * P + 2) * D + WIN]      # bf16
        b1c = cv_t[:, WIN:WIN + 1]
        ones_t = cpool.tile([65, D], bf16, tag="ones", name="ones")
        nc.vector.memset(ones_t[:], 1.0)

        half_tab = {0: tab_t[:, 0:NLO], 1: tab_t[:, NLO:NTAB]}

        # ---- gather + S build, two half streams ----
        # Pre-zero gather-pool slots once: skipped/garbage slots must never
        # hold NaN (NaN * 0 = NaN in the matmul); after this they only ever
        # hold previously gathered finite values.
        for tg in ("gL", "gH"):
            for _ in range(GBUFS):
                gw = gpool.tile([CHUNK, KG * CHUNK], bf16, tag=tg, name=tg)
                nc.vector.memset(gw[:], 0.0)

        gtiles = {}
        stiles = {}

        def ensure_batch(hf, g):
            if (hf, g) in gtiles:
                return
            tg = "gL" if hf == 0 else "gH"
            ix = mpool.tile([CHUNK, KG * CHUNK // 16], dt.int16,
                            tag="ix" + tg, name="ix" + tg)
            nc.sync.dma_start(
                out=ix[:],
                in_=idxd[hf][:, g * (KG * CHUNK // 16):(g + 1) * (KG * CHUNK // 16)])
            mdoff = mpool.tile([CHUNK, KG], bf16, tag="md" + tg,
                               name="md" + tg)
            nc.sync.dma_start(out=mdoff[:], in_=doffd[hf][:, g * KG:(g + 1) * KG])
            mval = mpool.tile([CHUNK, KG], bf16, tag="mv" + tg,
                              name="mv" + tg)
            nc.sync.dma_start(out=mval[:], in_=vald[hf][:, g * KG:(g + 1) * KG])
            gt = gpool.tile([CHUNK, KG * CHUNK], bf16, tag=tg, name=tg)
            nc.gpsimd.dma_gather(
                out_ap=gt[:].rearrange("p (c e) -> p c e", c=1),
                in_ap=half_tab[hf],
                idxs_ap=ix[:],
                num_idxs=KG * CHUNK,
                num_idxs_reg=int(nreg[hf][g]),
                elem_size=D,
                transpose=True,
                sbuf_tokens_per_rank=128,
                sbuf_free_dim_per_rank=256,
                single_packet=False,
            )
            eq = spool.tile([CHUNK, KG * WIN], bf16, tag="eq", name="eq",
                            bufs=2)
            st = spool.tile([CHUNK, KG * WIN], bf16, tag="s" + tg,
                            name="s" + tg)
            nc.vector.tensor_tensor(
                out=eq[:],
                in0=iota.unsqueeze(1).to_broadcast([CHUNK, KG, WIN]),
                in1=mdoff[:].unsqueeze(2).to_broadcast([CHUNK, KG, WIN]),
                op=mybir.AluOpType.is_equal,
            )
            nc.vector.tensor_tensor(
                out=st[:],
                in0=eq[:],
                in1=mval[:].unsqueeze(2).to_broadcast([CHUNK, KG, WIN]),
                op=mybir.AluOpType.mult,
            )
            gtiles[(hf, g)] = gt
            stiles[(hf, g)] = st

        chunk_c = [0, 0]
        n_evac = 0

        for b in range(NBANK):
            csl = slice(b * BANK, (b + 1) * BANK)
            ty = 0 if b < B0 // BANK else 1
            rows = work.tile([65, BANK], bf16, tag="rows", name="rows")
            zbs = []
            for m in range(P):
                agg = ps_agg.tile([D, BANK], f32, space="PSUM", tag="agg",
                                  name="agg")
                wsel = 3 * m + ty
                # ---- segment chunk list: (hf, g, cl, wl, j, tot) ----
                seg = []
                for wl in range(WPB):
                    w = b * WPB + wl
                    tot = int(counts2[0, m, w]) + int(counts2[1, m, w])
                    j = 0
                    for hf in range(2):
                        for _ in range(int(counts2[hf, m, w])):
                            g, cli = divmod(chunk_c[hf], KG)
                            seg.append((hf, g, cli, wl, j, tot))
                            chunk_c[hf] += 1
                            j += 1
                # ---- process in quads: Wmm x4 -> evac -> one-hot x4 ----
                for q0 in range(0, len(seg), 4):
                    quad = seg[q0:q0 + 4]
                    pst = ps_t.tile([D, BANK], f32, space="PSUM", tag="pst",
                                    name="pst")
                    for qi, (hf, g, cli, wl, j, tot) in enumerate(quad):
                        ensure_batch(hf, g)
                        nc.tensor.matmul(
                            out=pst[:, qi * D:(qi + 1) * D],
                            lhsT=gtiles[(hf, g)][:, cli * D:(cli + 1) * D],
                            rhs=wmat(wsel),
                            start=True, stop=True,
                        )
                    tsb = tpool.tile([D, BANK], bf16, tag="tsb", name="tsb")
                    nq = len(quad) * D
                    if n_evac % 2 == 0:
                        nc.scalar.copy(out=tsb[:, 0:nq], in_=pst[:, 0:nq])
                    else:
                        nc.vector.tensor_copy(tsb[:, 0:nq], pst[:, 0:nq])
                    n_evac += 1
                    for qi, (hf, g, cli, wl, j, tot) in enumerate(quad):
                        nc.tensor.matmul(
                            out=agg[:, wl * WIN:(wl + 1) * WIN],
                            lhsT=tsb[:, qi * D:(qi + 1) * D],
                            rhs=stiles[(hf, g)][:, cli * WIN:(cli + 1) * WIN],
                            start=(j == 0),
                            stop=False,
                        )
                        if j == tot - 1:
                            # seq_fts residual closes this window's group
                            wsl = slice(b * BANK + wl * WIN,
                                        b * BANK + (wl + 1) * WIN)
                            nc.tensor.matmul(
                                out=agg[:, wl * WIN:(wl + 1) * WIN],
                                lhsT=wmat(3 * m + 2), rhs=hT_t[:, wsl],
                                start=False, stop=True)
                # ---- PReLU(u + bfb) = max(u + bfb, a*u + a*bfb) ----
                bfb = cv_t[:, WIN + 2 + 4 * m + ty:WIN + 3 + 4 * m + ty]
                abfb = cv_t[:, WIN + 4 + 4 * m + ty:WIN + 5 + 4 * m + ty]
                t0 = work.tile([D, BANK], f32, tag="t0", name="t0", bufs=1)
                t1 = work.tile([D, BANK], f32, tag="t1", name="t1", bufs=1)
                nc.scalar.activation(t0[:], agg[:],
                                     mybir.ActivationFunctionType.Identity,
                                     bias=bfb, scale=1.0)
                nc.scalar.activation(t1[:], agg[:],
                                     mybir.ActivationFunctionType.Identity,
                                     bias=abfb, scale=float(alphas[m]))
                zbf = work.tile([D, BANK], f32, tag=f"zf{m}", name=f"zf{m}")
                nc.vector.tensor_tensor(out=zbf[:], in0=t0[:], in1=t1[:],
                                        op=mybir.AluOpType.max)
                zb = work.tile([D, BANK], bf16, tag=f"zb{m}", name=f"zb{m}")
                nc.scalar.copy(out=zb[:], in_=zbf[:])
                zbs.append(zbf)
                # ---- attention score -> exp, stored at partition 32m ----
                aps = ps_attn.tile([D, BANK], f32, space="PSUM", tag="at",
                                   name="at")
                nc.tensor.matmul(out=aps[:], lhsT=attW1T, rhs=zb[:],
                                 start=True, stop=True)
                th = work.tile([D, BANK], bf16, tag="tanh", name="tanh")
                nc.scalar.activation(th[:], aps[:],
                                     mybir.ActivationFunctionType.Tanh,
                                     bias=b1c, scale=1.0)
                sps = ps_attn.tile([D, BANK], f32, space="PSUM", tag="at",
                                   name="at")
                nc.tensor.matmul(out=sps[0:1, :], lhsT=w2col, rhs=th[:],
                                 start=True, stop=True)
                nc.scalar.activation(rows[32 * m:32 * m + 1, :], sps[0:1, :],
                                     mybir.ActivationFunctionType.Exp)

            # ---- softmax-weighted combine for this bank ----
            bsum = ps_bc.tile([D, BANK], f32, space="PSUM", tag="bc",
                              name="bc")
            for m in range(P):
                nc.tensor.matmul(out=bsum[0:1, :],
                                 lhsT=ones_t[32 * m:32 * m + 1, 0:1],
                                 rhs=rows[32 * m:32 * m + 1, :],
                                 start=(m == 0), stop=(m == P - 1))
            rc = work.tile([1, BANK], f32, tag="rc", name="rc", bufs=1)
            nc.vector.reciprocal(out=rc[:], in_=bsum[0:1, :])
            rc16 = work.tile([1, BANK], bf16, tag="rc16", name="rc16", bufs=1)
            nc.vector.tensor_copy(rc16[:], rc[:])
            acc = work.tile([D, BANK], f32, tag="acc", name="acc")
            tmp = work.tile([D, BANK], f32, tag="tmp", name="tmp", bufs=1)
            for m in range(P):
                ebm = ps_bc.tile([D, BANK], f32, space="PSUM", tag="bc",
                                 name="bc")
                nc.tensor.matmul(out=ebm[:],
                                 lhsT=ones_t[32 * m:32 * m + 1, :],
                                 rhs=rows[32 * m:32 * m + 1, :],
                                 start=True, stop=True)
                dst = acc if m == 0 else tmp
                nc.vector.tensor_tensor(out=dst[:], in0=zbs[m][:],
                                        in1=ebm[:], op=mybir.AluOpType.mult)
                if m > 0:
                    nc.vector.tensor_tensor(out=acc[:], in0=acc[:],
                                            in1=tmp[:],
                                            op=mybir.AluOpType.add)
            rcb = ps_bc.tile([D, BANK], f32, space="PSUM", tag="bc",
                             name="bc")
            nc.tensor.matmul(out=rcb[:], lhsT=ones_t[0:1, :],
                             rhs=rc16[:], start=True, stop=True)
            nc.vector.tensor_tensor(out=acc[:], in0=acc[:], in1=rcb[:],
                                    op=mybir.AluOpType.mult)
            hTs = work.tile([D, BANK], f32, tag="hTs", name="hTs", bufs=1)
            nc.scalar.copy(out=hTs[:], in_=hT_t[:, csl])
            nc.vector.tensor_tensor(out=acc[:], in0=acc[:], in1=hTs[:],
                                    op=mybir.AluOpType.add)
            nc.sync.dma_start(out=outd[:, csl], in_=acc[:])

        assert chunk_c[0] == int(counts2[0].sum())
        assert chunk_c[1] == int(counts2[1].sum())

    nc.compile()
    return nc


# ---------------------------------------------------------------------------
# entry point
# ---------------------------------------------------------------------------

def kernel(h, edge_rows, edge_cols, edge_vals, node_type,
           W_fc, prelu_a, Wg, bg, Wb, bb, film_bias,
           att_W1, att_b1, att_w2, _run_opts=None):
    _ensure_path()
    import ml_dtypes
    from concourse import bass_utils

    BF = ml_dtypes.bfloat16

    h = np.asarray(h, dtype=F32)
    edge_rows = np.asarray(edge_rows)
    edge_cols = np.asarray(edge_cols)
    edge_vals = np.asarray(edge_vals, dtype=F32)
    node_type = np.asarray(node_type)

    cfg, per_core = _plan(h, edge_rows, edge_cols, edge_vals, node_type)
    wmats16, cvec = _pack_weights(cfg, np.asarray(W_fc), np.asarray(prelu_a),
                                  np.asarray(Wg), np.asarray(bg),
                                  np.asarray(Wb), np.asarray(bb),
                                  np.asarray(film_bias), np.asarray(att_W1),
                                  np.asarray(att_b1), np.asarray(att_w2))

    nc = _build_program(cfg, np.asarray(prelu_a, dtype=F32))

    N = cfg["N"]
    npc = cfg["npc"]
    B0 = cfg["B0"]
    NCOL = cfg["NCOL"]
    NTAB = _round_up(N, 128)

    # node table, token-striped: tab[p, r*128:(r+1)*128] = bf16(h[r*128+p])
    hp = np.zeros((NTAB, D), dtype=F32)
    hp[:N] = h
    tab = np.ascontiguousarray(
        _to_bf16_bits(hp).reshape(NTAB // 128, 128, D).transpose(1, 0, 2)
        .reshape(128, NTAB)).view(BF)

    in_maps = []
    for c in range(N_CORES):
        pc = per_core[c]
        hT_own = np.zeros((D, NCOL), dtype=F32)
        own = h[c * npc:(c + 1) * npc]       # [npc, D]
        srt = own[pc["perm"]]                 # type-sorted rows
        n0 = pc["n0"]
        hT_own[:, :n0] = srt[:n0].T
        hT_own[:, B0:B0 + (npc - n0)] = srt[n0:].T
        im = {
            "tab": tab,
            "hT": _to_bf16_bits(hT_own).view(BF),
            "wmats": wmats16.view(BF),
            "cvec": cvec,
        }
        for tag in ("L", "H"):
            for nm in ("idx", "doff", "val"):
                arr = pc[nm + tag]
                if arr.shape[1] == 0:  # empty stream: dram tensor padded to 1
                    arr = np.zeros(
                        (CHUNK, 1),
                        dtype=np.int16 if nm == "idx" else F32)
                    if nm == "idx":
                        arr -= 1
                if nm in ("doff", "val"):
                    arr = _to_bf16_bits(arr).view(BF)
                im[nm + tag] = arr
        in_maps.append(im)

    run_kwargs = dict(_run_opts or {})
    run_kwargs.pop("_result", None)
    res = bass_utils.run_bass_kernel_spmd(
        nc, in_maps, core_ids=list(range(N_CORES)), **run_kwargs
    )

    out = np.empty((N, D), dtype=F32)
    for c in range(N_CORES):
        pc = per_core[c]
        n0 = pc["n0"]
        zT = res.results[c]["outT"]           # [D, NCOL]
        real = np.concatenate(
            [zT[:, :n0], zT[:, B0:B0 + (npc - n0)]], axis=1
        ).T                                    # [npc, D] sorted order
        shard = np.empty((npc, D), dtype=F32)
        shard[pc["perm"]] = real
        out[c * npc:(c + 1) * npc] = shard
    if isinstance(_run_opts, dict):
        _run_opts["_result"] = res
    return out


# revision 27
# speedup vs baseline: 1.0318x; 1.0071x over previous
"""MGNN (gnn_message_passing) Trainium2 kernel — v2.

Strategy (8 NeuronCores, destination-sharded, no collectives):
  - Each core owns N/8 = 6250 destination nodes; host partitions edge lists
    by destination row, type-sorts destinations, pads per-(metapath, window)
    chunk groups so all 8 cores run one SPMD program (as v1).
  - NEW: the full node table h is kept RESIDENT IN SBUF as bf16 (12.8 MB,
    token-striped layout), loaded once sequentially. Per-edge source rows are
    then gathered SBUF->SBUF via transpose-mode dma_gather — removing the
    ~120 MB/core of latency-bound random HBM reads that dominated v1.
  - The transpose-gather output is feature-major [D, edges]; the mandatory
    re-transpose to edge-major is fused with the FiLM weight multiply:
        T[e, j] = sum_d G[d, e] * (diag(gamma_ty) W_m)^T[d, j]
    one matmul per 128-edge chunk (PSUM), evacuated to SBUF in 512-col
    batches alternating scalar/vector engines.
  - Segment-sum via one-hot matmul per 64-dest window (S built on DVE in
    bf16), accumulating z_pre^T directly in PSUM; the seq_fts residual
    (W_m h^T) is matmul-accumulated into the same PSUM bank.
  - Loop order is bank-major: per bank all 3 metapath z's stay in SBUF
    (bf16), softmax over metapaths is computed in-place via PE broadcast
    tricks (no z spill to HBM, no node-major reshape DMAs).
  - All matmul operands are bf16 (PSUM accumulation f32).
"""

import math
import os

import numpy as np


def _ensure_path():
    try:
        import concourse  # noqa: F401
    except ImportError:
        import sys

        for p in ("/opt/trn_rl_repo", "/root/.axon_site/_ro/trn_rl_repo"):
            if os.path.isdir(p) and p not in sys.path:
                sys.path.insert(0, p)


# ---------------------------------------------------------------------------
# configuration
# ---------------------------------------------------------------------------

N_CORES = 8
D = 128          # hidden dim (= partition count)
CHUNK = 128      # edges per matmul chunk (contraction dim)
WIN = 64         # destinations per one-hot window (S width)
KG = 16          # chunks per dma_gather instruction
BANK = 512       # psum bank width (f32 elems) = 8 windows
WPB = BANK // WIN
NLO = 32768      # token split for int16 gather indices (256 ranks)

F32 = np.float32
I32 = np.int32


def _round_up(x, m):
    return (x + m - 1) // m * m


def _to_bf16_bits(x):
    x = np.ascontiguousarray(x, dtype=np.float32)
    return ((x.view(np.uint32) + 0x8000) >> 16).astype(np.uint16)


# ---------------------------------------------------------------------------
# host-side planning
# ---------------------------------------------------------------------------

def _plan(h, edge_rows, edge_cols, edge_vals, node_type):
    """Shard by destination, type-sort each shard, build padded chunk plan.

    Chunk groups are keyed (half, metapath, window) and laid out in the two
    half streams in CONSUMPTION order: bank-major, then metapath, then
    window, then half. Returns (cfg, per_core).
    """
    N = h.shape[0]
    P = edge_rows.shape[0]
    npc = N // N_CORES
    assert npc * N_CORES == N
    assert NLO <= 32768 and (N - NLO) <= 32767

    # --- per-core destination shards, sorted by node_type (stable) ---
    shards = []
    for c in range(N_CORES):
        own = slice(c * npc, (c + 1) * npc)
        t = node_type[own]
        perm = np.argsort(t, kind="stable")  # sorted-rank -> original local id
        n0 = int((t == 0).sum())
        shards.append({"perm": perm, "n0": n0})

    max_n0 = max(s["n0"] for s in shards)
    max_n1 = max(npc - s["n0"] for s in shards)
    B0 = _round_up(max(max_n0, 1), BANK)
    NCOL = B0 + _round_up(max(max_n1, 1), BANK)
    NBANK = NCOL // BANK
    NWIN = NCOL // WIN

    # padded-column map per core: local node id -> column
    for s in shards:
        inv = np.empty(npc, dtype=np.int64)
        inv[s["perm"]] = np.arange(npc)  # original local id -> sorted rank
        col = np.where(inv < s["n0"], inv, B0 + (inv - s["n0"]))
        s["colmap"] = col

    # --- edge bucketing by (core, metapath, half, window) ---
    edge_data = [[None] * P for _ in range(N_CORES)]
    hist = np.zeros((2, N_CORES, P, NWIN), dtype=np.int64)
    for c in range(N_CORES):
        base = c * npc
        for m in range(P):
            er = edge_rows[m]
            mask = (er >= base) & (er < base + npc)
            dl = shards[c]["colmap"][er[mask] - base]
            cs = edge_cols[m][mask].astype(np.int64)
            vs = edge_vals[m][mask].astype(F32)
            half = (cs >= NLO).astype(np.int64)
            # sort by (window, half) so each (w, half) group is contiguous
            key = (dl // WIN) * 2 + half
            order = np.argsort(key, kind="stable")
            dl = dl[order]
            cs = cs[order]
            vs = vs[order]
            half = half[order]
            w = dl // WIN
            for hf in range(2):
                hist[hf, c, m] += np.bincount(w[half == hf], minlength=NWIN)
            edge_data[c][m] = (dl, cs, vs, w, half)

    cl = np.maximum(1, -(-hist[0].max(axis=0) // CHUNK))   # [P, NWIN] lo
    ch = -(-hist[1].max(axis=0) // CHUNK)                  # [P, NWIN] hi
    counts2 = np.stack([cl, ch])                           # [2, P, NWIN]
    nch = [int(cl.sum()), int(ch.sum())]
    nch_pad = [_round_up(max(x, 1), KG) for x in nch]

    # chunk slot base per (half, m, w), stream order = (bank, m, wl)
    base_slot = np.zeros((2, P, NWIN), dtype=np.int64)
    for hf in range(2):
        acc = 0
        for b in range(NBANK):
            for m in range(P):
                for wl in range(WPB):
                    w = b * WPB + wl
                    base_slot[hf, m, w] = acc
                    acc += int(counts2[hf, m, w])
        assert acc == nch[hf]

    per_core = []
    for c in range(N_CORES):
        streams = []
        for hf in range(2):
            nitems = nch_pad[hf] * CHUNK
            streams.append({
                "idx": np.full(nitems, -1, dtype=np.int64),
                "doff": np.zeros(nitems, dtype=F32),
                "val": np.zeros(nitems, dtype=F32),
            })
            # in-plan chunks: pad items default to row 0 / val 0
            ninplan = nch[hf] * CHUNK
            streams[hf]["idx"][:ninplan] = 0
        for m in range(P):
            dl, cs, vs, w, half = edge_data[c][m]
            for hf in range(2):
                sel = half == hf
                wm_ = w[sel]
                starts = np.searchsorted(wm_, np.arange(NWIN))
                rank = np.arange(len(wm_)) - starts[wm_]
                slot = base_slot[hf, m, wm_] * CHUNK + rank
                st = streams[hf]
                st["idx"][slot] = cs[sel] - hf * NLO
                st["doff"][slot] = (dl[sel] - wm_ * WIN).astype(F32)
                st["val"][slot] = vs[sel]
        pc = {"perm": shards[c]["perm"], "n0": shards[c]["n0"]}
        for hf, tag in ((0, "L"), (1, "H")):
            st = streams[hf]
            # idx items wrapped in 16 partitions, replicated to 128
            iw = np.ascontiguousarray(
                st["idx"].reshape(-1, 16).T).astype(np.int16)   # [16, items/16]
            pc["idx" + tag] = np.tile(iw, (8, 1))               # [128, items/16]
            pc["doff" + tag] = np.ascontiguousarray(
                st["doff"].reshape(-1, CHUNK).T)                # [128, nch_pad]
            pc["val" + tag] = np.ascontiguousarray(
                st["val"].reshape(-1, CHUNK).T)
        per_core.append(pc)

    # per-gather-batch valid index counts (same for all cores by construction)
    nreg = []
    for hf in range(2):
        ninplan = nch[hf]
        nb = nch_pad[hf] // KG
        r = []
        for g in range(nb):
            lo_c = g * KG
            r.append(max(0, min(ninplan - lo_c, KG)) * CHUNK)
        nreg.append(r)

    cfg = {
        "N": N,
        "P": P,
        "npc": npc,
        "B0": B0,
        "NCOL": NCOL,
        "NBANK": NBANK,
        "NWIN": NWIN,
        "counts2": counts2,
        "nch_pad": nch_pad,
        "nreg": nreg,
    }
    return cfg, per_core


def _pack_weights(cfg, W_fc, prelu_a, Wg, bg, Wb, bb, film_bias,
                  att_W1, att_b1, att_w2):
    """Pack small weights: bf16 matmul blob + f32 constant vector."""
    P = cfg["P"]
    # wmats (bf16): per meta [W0T, W1T, WfcT], then att_W1T, a w2-column
    # block, then a bf16 iota row  -> [128, (3P+2)*128 + WIN]
    blocks = []
    for m in range(P):
        g0 = (Wg[m][:, 0] + bg[m]).astype(F32)  # [D]
        g1 = (Wg[m][:, 1] + bg[m]).astype(F32)
        WT = W_fc[m].T.astype(F32)              # [fi, fo]
        blocks += [WT * g0[None, :], WT * g1[None, :], WT]
    blocks.append(att_W1.T.astype(F32))          # lhsT[d, hid]
    extra = np.zeros((D, D), dtype=F32)
    extra[:, 0] = att_w2.astype(F32)             # w2 column (lhsT [128,1])
    blocks.append(extra)
    blocks.append(np.tile(np.arange(WIN, dtype=F32), (D, 1)))  # iota16
    wmats = np.concatenate(blocks, axis=1)
    wmats16 = _to_bf16_bits(wmats)               # uint16 view

    # cvec f32 [128, WIN+16]: b1, per-meta (bfb0, bfb1, a*bfb0, a*bfb1)
    cvec = np.zeros((D, WIN + 16), dtype=F32)
    cvec[:, WIN] = att_b1.astype(F32)
    for m in range(P):
        a = float(prelu_a[m])
        bfb0 = (Wb[m][:, 0] + bb[m] + film_bias[m]).astype(F32)
        bfb1 = (Wb[m][:, 1] + bb[m] + film_bias[m]).astype(F32)
        cvec[:, WIN + 2 + 4 * m] = bfb0
        cvec[:, WIN + 3 + 4 * m] = bfb1
        cvec[:, WIN + 4 + 4 * m] = a * bfb0
        cvec[:, WIN + 5 + 4 * m] = a * bfb1
    return wmats16, cvec


# ---------------------------------------------------------------------------
# device program
# ---------------------------------------------------------------------------

def _build_program(cfg, alphas):
    _ensure_path()
    import concourse.bass as bass  # noqa: F401
    import concourse.tile as tile
    from concourse import bacc, mybir

    P = cfg["P"]
    NCOL = cfg["NCOL"]
    NBANK = cfg["NBANK"]
    B0 = cfg["B0"]
    counts2 = cfg["counts2"]
    nch_pad = cfg["nch_pad"]
    nreg = cfg["nreg"]
    N = cfg["N"]
    NRANK = _round_up(N, 128) // 128
    NTAB = NRANK * 128
    dt = mybir.dt
    f32 = dt.float32
    bf16 = dt.bfloat16

    nc = bacc.Bacc(
        "TRN2",
        target_bir_lowering=False,
        debug=False,
        enable_asserts=False,
        num_devices=N_CORES,
    )

    tabd = nc.dram_tensor("tab", [D, NTAB], bf16, kind="ExternalInput").ap()
    hTd = nc.dram_tensor("hT", [D, NCOL], bf16, kind="ExternalInput").ap()
    idxd = {}
    doffd = {}
    vald = {}
    for hf, tag in ((0, "L"), (1, "H")):
        ni = max(nch_pad[hf] * CHUNK // 16, 1)
        idxd[hf] = nc.dram_tensor(f"idx{tag}", [CHUNK, ni], dt.int16,
                                  kind="ExternalInput").ap()
        nch1 = max(nch_pad[hf], 1)
        doffd[hf] = nc.dram_tensor(f"doff{tag}", [CHUNK, nch1], bf16,
                                   kind="ExternalInput").ap()
        vald[hf] = nc.dram_tensor(f"val{tag}", [CHUNK, nch1], bf16,
                                  kind="ExternalInput").ap()
    wmatsd = nc.dram_tensor("wmats", [D, (3 * P + 2) * D + WIN], bf16,
                            kind="ExternalInput").ap()
    cvecd = nc.dram_tensor("cvec", [D, WIN + 16], f32, kind="ExternalInput").ap()
    outd = nc.dram_tensor("outT", [D, NCOL], f32, kind="ExternalOutput").ap()

    GBUFS = 3

    with tile.TileContext(nc) as tc, tc.tile_pool(name="const", bufs=1) as cpool, \
            tc.tile_pool(name="gpool", bufs=GBUFS) as gpool, \
            tc.tile_pool(name="spool", bufs=3) as spool, \
            tc.tile_pool(name="mpool", bufs=3) as mpool, \
            tc.tile_pool(name="tpool", bufs=3) as tpool, \
            tc.tile_pool(name="work", bufs=2) as work, \
            tc.tile_pool(name="ps_t", bufs=2, space="PSUM") as ps_t, \
            tc.tile_pool(name="ps_agg", bufs=2, space="PSUM") as ps_agg, \
            tc.tile_pool(name="ps_attn", bufs=2, space="PSUM") as ps_attn, \
            tc.tile_pool(name="ps_bc", bufs=2, space="PSUM") as ps_bc:

        # ---- constants / resident inputs ----
        tab_t = cpool.tile([D, NTAB], bf16, tag="tab", name="tab")
        nc.sync.dma_start(out=tab_t[:], in_=tabd)
        hT_t = cpool.tile([D, NCOL], bf16, tag="hT", name="hT")
        nc.sync.dma_start(out=hT_t[:], in_=hTd)
        wm_t = cpool.tile([D, (3 * P + 2) * D + WIN], bf16, tag="wm",
                          name="wm")
        nc.sync.dma_start(out=wm_t[:], in_=wmatsd)
        cv_t = cpool.tile([D, WIN + 16], f32, tag="cv", name="cv")
        nc.sync.dma_start(out=cv_t[:], in_=cvecd)

        def wmat(i):  # [128,128] bf16 lhsT block i
            return wm_t[:, i * D:(i + 1) * D]

        attW1T = wmat(3 * P)
        w2col = wm_t[:, (3 * P + 1) * D:(3 * P + 1) * D + 1]       # [128,1]
        iota = wm_t[:, (3 * P + 2) * D:(3 * P + 2) * D + WIN]      # bf16
        b1c = cv_t[:, WIN:WIN + 1]
        ones_t = cpool.tile([65, D], bf16, tag="ones", name="ones")
        nc.vector.memset(ones_t[:], 1.0)

        half_tab = {0: tab_t[:, 0:NLO], 1: tab_t[:, NLO:NTAB]}

        # ---- gather + S build, two half streams ----
        # Pre-zero gather-pool slots once: skipped/garbage slots must never
        # hold NaN (NaN * 0 = NaN in the matmul); after this they only ever
        # hold previously gathered finite values.
        for tg in ("gL", "gH"):
            for _ in range(GBUFS):
                gw = gpool.tile([CHUNK, KG * CHUNK], bf16, tag=tg, name=tg)
                nc.vector.memset(gw[:], 0.0)

        gtiles = {}
        stiles = {}

        def ensure_batch(hf, g):
            if (hf, g) in gtiles:
                return
            tg = "gL" if hf == 0 else "gH"
            ix = mpool.tile([CHUNK, KG * CHUNK // 16], dt.int16,
                            tag="ix" + tg, name="ix" + tg)
            nc.sync.dma_start(
                out=ix[:],
                in_=idxd[hf][:, g * (KG * CHUNK // 16):(g + 1) * (KG * CHUNK // 16)])
            mdoff = mpool.tile([CHUNK, KG], bf16, tag="md" + tg,
                               name="md" + tg)
            nc.sync.dma_start(out=mdoff[:], in_=doffd[hf][:, g * KG:(g + 1) * KG])
            mval = mpool.tile([CHUNK, KG], bf16, tag="mv" + tg,
                              name="mv" + tg)
            nc.sync.dma_start(out=mval[:], in_=vald[hf][:, g * KG:(g + 1) * KG])
            gt = gpool.tile([CHUNK, KG * CHUNK], bf16, tag=tg, name=tg)
            nc.gpsimd.dma_gather(
                out_ap=gt[:].rearrange("p (c e) -> p c e", c=1),
                in_ap=half_tab[hf],
                idxs_ap=ix[:],
                num_idxs=KG * CHUNK,
                num_idxs_reg=int(nreg[hf][g]),
                elem_size=D,
                transpose=True,
                sbuf_tokens_per_rank=128,
                sbuf_free_dim_per_rank=256,
                single_packet=False,
            )
            eq = spool.tile([CHUNK, KG * WIN], bf16, tag="eq", name="eq",
                            bufs=2)
            st = spool.tile([CHUNK, KG * WIN], bf16, tag="s" + tg,
                            name="s" + tg)
            nc.vector.tensor_tensor(
                out=eq[:],
                in0=iota.unsqueeze(1).to_broadcast([CHUNK, KG, WIN]),
                in1=mdoff[:].unsqueeze(2).to_broadcast([CHUNK, KG, WIN]),
                op=mybir.AluOpType.is_equal,
            )
            nc.vector.tensor_tensor(
                out=st[:],
                in0=eq[:],
                in1=mval[:].unsqueeze(2).to_broadcast([CHUNK, KG, WIN]),
                op=mybir.AluOpType.mult,
            )
            gtiles[(hf, g)] = gt
            stiles[(hf, g)] = st

        # ------------------------------------------------------------------
        # build the full schedule (segments = (bank, metapath); quads of 4
        # chunks), then emit with a software pipeline so the PE never waits
        # on the cross-engine evac round-trip:
        #   step k: Wmm(quad k) | evac(quad k-1) | one-hot(quad k-2)
        # ------------------------------------------------------------------
        chunk_c = [0, 0]
        sched = []
        for b in range(NBANK):
            ty = 0 if b < B0 // BANK else 1
            for m in range(P):
                seg = []
                for wl in range(WPB):
                    w = b * WPB + wl
                    tot = int(counts2[0, m, w]) + int(counts2[1, m, w])
                    j = 0
                    for hf in range(2):
                        for _ in range(int(counts2[hf, m, w])):
                            g, cli = divmod(chunk_c[hf], KG)
                            seg.append((hf, g, cli, wl, j, tot))
                            chunk_c[hf] += 1
                            j += 1
                quads = [seg[i:i + 4] for i in range(0, len(seg), 4)]
                sched.append({"b": b, "m": m, "ty": ty, "quads": quads,
                              "agg": None})
        assert chunk_c[0] == int(counts2[0].sum())
        assert chunk_c[1] == int(counts2[1].sum())

        flat = []
        for si, s in enumerate(sched):
            for qi_, q in enumerate(s["quads"]):
                flat.append((si, q, qi_ == len(s["quads"]) - 1))

        bank_rows = {}
        bank_zbs = {}

        def post_bank(b, rows, zbs, csl):
            # softmax-weighted combine: out = (sum_m e_m z_m)/(sum_m e_m) + h
            bsum = ps_bc.tile([D, BANK], f32, space="PSUM", tag="bc",
                              name="bc")
            for m in range(P):
                nc.tensor.matmul(out=bsum[0:1, :],
                                 lhsT=ones_t[32 * m:32 * m + 1, 0:1],
                                 rhs=rows[32 * m:32 * m + 1, :],
                                 start=(m == 0), stop=(m == P - 1))
            rc = work.tile([1, BANK], f32, tag="rc", name="rc", bufs=1)
            nc.vector.reciprocal(out=rc[:], in_=bsum[0:1, :])
            rc16 = work.tile([1, BANK], bf16, tag="rc16", name="rc16", bufs=1)
            nc.vector.tensor_copy(rc16[:], rc[:])
            acc = work.tile([D, BANK], f32, tag="acc", name="acc")
            tmp = work.tile([D, BANK], f32, tag="tmp", name="tmp", bufs=2)
            for m in range(P):
                ebm = ps_bc.tile([D, BANK], f32, space="PSUM", tag="bc",
                                 name="bc")
                nc.tensor.matmul(out=ebm[:],
                                 lhsT=ones_t[32 * m:32 * m + 1, :],
                                 rhs=rows[32 * m:32 * m + 1, :],
                                 start=True, stop=True)
                dst = acc if m == 0 else tmp
                nc.vector.tensor_tensor(out=dst[:], in0=zbs[m][:],
                                        in1=ebm[:], op=mybir.AluOpType.mult)
                if m > 0:
                    nc.vector.tensor_tensor(out=acc[:], in0=acc[:],
                                            in1=tmp[:],
                                            op=mybir.AluOpType.add)
            rcb = ps_bc.tile([D, BANK], f32, space="PSUM", tag="bc",
                             name="bc")
            nc.tensor.matmul(out=rcb[:], lhsT=ones_t[0:1, :],
                             rhs=rc16[:], start=True, stop=True)
            nc.vector.tensor_tensor(out=acc[:], in0=acc[:], in1=rcb[:],
                                    op=mybir.AluOpType.mult)
            hTs = work.tile([D, BANK], f32, tag="hTs", name="hTs", bufs=2)
            nc.scalar.copy(out=hTs[:], in_=hT_t[:, csl])
            nc.vector.tensor_tensor(out=acc[:], in0=acc[:], in1=hTs[:],
                                    op=mybir.AluOpType.add)
            nc.sync.dma_start(out=outd[:, csl], in_=acc[:])

        def post_segment(s):
            b, m, ty = s["b"], s["m"], s["ty"]
            agg = s["agg"]
            csl = slice(b * BANK, (b + 1) * BANK)
            if b not in bank_rows:
                bank_rows[b] = work.tile([65, BANK], bf16, tag="rows",
                                         name="rows")
                bank_zbs[b] = []
            rows = bank_rows[b]
            # PReLU(u + bfb) = max(u + bfb, a*u + a*bfb)
            bfb = cv_t[:, WIN + 2 + 4 * m + ty:WIN + 3 + 4 * m + ty]
            abfb = cv_t[:, WIN + 4 + 4 * m + ty:WIN + 5 + 4 * m + ty]
            t0 = work.tile([D, BANK], f32, tag="t0", name="t0", bufs=2)
            t1 = work.tile([D, BANK], f32, tag="t1", name="t1", bufs=2)
            nc.scalar.activation(t0[:], agg[:],
                                 mybir.ActivationFunctionType.Identity,
                                 bias=bfb, scale=1.0)
            nc.scalar.activation(t1[:], agg[:],
                                 mybir.ActivationFunctionType.Identity,
                                 bias=abfb, scale=float(alphas[m]))
            zbf = work.tile([D, BANK], f32, tag=f"zf{m}", name=f"zf{m}")
            nc.vector.tensor_tensor(out=zbf[:], in0=t0[:], in1=t1[:],
                                    op=mybir.AluOpType.max)
            zb = work.tile([D, BANK], bf16, tag=f"zb{m}", name=f"zb{m}")
            nc.scalar.copy(out=zb[:], in_=zbf[:])
            bank_zbs[b].append(zbf)
            # attention score -> exp, stored at partition 32m
            aps = ps_attn.tile([D, BANK], f32, space="PSUM", tag="at",
                               name="at")
            nc.tensor.matmul(out=aps[:], lhsT=attW1T, rhs=zb[:],
                             start=True, stop=True)
            th = work.tile([D, BANK], bf16, tag="tanh", name="tanh")
            nc.scalar.activation(th[:], aps[:],
                                 mybir.ActivationFunctionType.Tanh,
                                 bias=b1c, scale=1.0)
            sps = ps_attn.tile([D, BANK], f32, space="PSUM", tag="at",
                               name="at")
            nc.tensor.matmul(out=sps[0:1, :], lhsT=w2col, rhs=th[:],
                             start=True, stop=True)
            nc.scalar.activation(rows[32 * m:32 * m + 1, :], sps[0:1, :],
                                 mybir.ActivationFunctionType.Exp)
            if m == P - 1:
                post_bank(b, rows, bank_zbs.pop(b), csl)
                del bank_rows[b]

        n_evac = 0
        state = {}
        NQ = len(flat)
        for k in range(NQ + 2):
            if k < NQ:
                si, quad, last = flat[k]
                s = sched[si]
                wsel = 3 * s["m"] + s["ty"]
                pst = ps_t.tile([D, BANK], f32, space="PSUM", tag="pst",
                                name="pst")
                for qi, (hf, g, cli, wl, j, tot) in enumerate(quad):
                    ensure_batch(hf, g)
                    nc.tensor.matmul(
                        out=pst[:, qi * D:(qi + 1) * D],
                        lhsT=gtiles[(hf, g)][:, cli * D:(cli + 1) * D],
                        rhs=wmat(wsel),
                        start=True, stop=True,
                    )
                state[k] = [pst, None]
            if k >= 1 and k - 1 < NQ:
                pst = state[k - 1][0]
                _, quad, _ = flat[k - 1]
                tsb = tpool.tile([D, BANK], bf16, tag="tsb", name="tsb")
                nq = len(quad) * D
                if n_evac % 2 == 0:
                    nc.scalar.copy(out=tsb[:, 0:nq], in_=pst[:, 0:nq])
                else:
                    nc.vector.tensor_copy(tsb[:, 0:nq], pst[:, 0:nq])
                n_evac += 1
                state[k - 1][1] = tsb
            if k >= 2:
                si, quad, last = flat[k - 2]
                s = sched[si]
                tsb = state.pop(k - 2)[1]
                if s["agg"] is None:
                    s["agg"] = ps_agg.tile([D, BANK], f32, space="PSUM",
                                           tag="agg", name="agg")
                agg = s["agg"]
                b = s["b"]
                for qi, (hf, g, cli, wl, j, tot) in enumerate(quad):
                    nc.tensor.matmul(
                        out=agg[:, wl * WIN:(wl + 1) * WIN],
                        lhsT=tsb[:, qi * D:(qi + 1) * D],
                        rhs=stiles[(hf, g)][:, cli * WIN:(cli + 1) * WIN],
                        start=(j == 0),
                        stop=False,
                    )
                    if j == tot - 1:
                        # seq_fts residual closes this window's group
                        wsl = slice(b * BANK + wl * WIN,
                                    b * BANK + (wl + 1) * WIN)
                        nc.tensor.matmul(
                            out=agg[:, wl * WIN:(wl + 1) * WIN],
                            lhsT=wmat(3 * s["m"] + 2), rhs=hT_t[:, wsl],
                            start=False, stop=True)
                if last:
                    post_segment(s)

    nc.compile()
    return nc


# ---------------------------------------------------------------------------
# entry point
# ---------------------------------------------------------------------------

def kernel(h, edge_rows, edge_cols, edge_vals, node_type,
           W_fc, prelu_a, Wg, bg, Wb, bb, film_bias,
           att_W1, att_b1, att_w2, _run_opts=None):
    _ensure_path()
    import ml_dtypes
    from concourse import bass_utils

    BF = ml_dtypes.bfloat16

    h = np.asarray(h, dtype=F32)
    edge_rows = np.asarray(edge_rows)
    edge_cols = np.asarray(edge_cols)
    edge_vals = np.asarray(edge_vals, dtype=F32)
    node_type = np.asarray(node_type)

    cfg, per_core = _plan(h, edge_rows, edge_cols, edge_vals, node_type)
    wmats16, cvec = _pack_weights(cfg, np.asarray(W_fc), np.asarray(prelu_a),
                                  np.asarray(Wg), np.asarray(bg),
                                  np.asarray(Wb), np.asarray(bb),
                                  np.asarray(film_bias), np.asarray(att_W1),
                                  np.asarray(att_b1), np.asarray(att_w2))

    nc = _build_program(cfg, np.asarray(prelu_a, dtype=F32))

    N = cfg["N"]
    npc = cfg["npc"]
    B0 = cfg["B0"]
    NCOL = cfg["NCOL"]
    NTAB = _round_up(N, 128)

    # node table, token-striped: tab[p, r*128:(r+1)*128] = bf16(h[r*128+p])
    hp = np.zeros((NTAB, D), dtype=F32)
    hp[:N] = h
    tab = np.ascontiguousarray(
        _to_bf16_bits(hp).reshape(NTAB // 128, 128, D).transpose(1, 0, 2)
        .reshape(128, NTAB)).view(BF)

    in_maps = []
    for c in range(N_CORES):
        pc = per_core[c]
        hT_own = np.zeros((D, NCOL), dtype=F32)
        own = h[c * npc:(c + 1) * npc]       # [npc, D]
        srt = own[pc["perm"]]                 # type-sorted rows
        n0 = pc["n0"]
        hT_own[:, :n0] = srt[:n0].T
        hT_own[:, B0:B0 + (npc - n0)] = srt[n0:].T
        im = {
            "tab": tab,
            "hT": _to_bf16_bits(hT_own).view(BF),
            "wmats": wmats16.view(BF),
            "cvec": cvec,
        }
        for tag in ("L", "H"):
            for nm in ("idx", "doff", "val"):
                arr = pc[nm + tag]
                if arr.shape[1] == 0:  # empty stream: dram tensor padded to 1
                    arr = np.zeros(
                        (CHUNK, 1),
                        dtype=np.int16 if nm == "idx" else F32)
                    if nm == "idx":
                        arr -= 1
                if nm in ("doff", "val"):
                    arr = _to_bf16_bits(arr).view(BF)
                im[nm + tag] = arr
        in_maps.append(im)

    run_kwargs = dict(_run_opts or {})
    run_kwargs.pop("_result", None)
    res = bass_utils.run_bass_kernel_spmd(
        nc, in_maps, core_ids=list(range(N_CORES)), **run_kwargs
    )

    out = np.empty((N, D), dtype=F32)
    for c in range(N_CORES):
        pc = per_core[c]
        n0 = pc["n0"]
        zT = res.results[c]["outT"]           # [D, NCOL]
        real = np.concatenate(
            [zT[:, :n0], zT[:, B0:B0 + (npc - n0)]], axis=1
        ).T                                    # [npc, D] sorted order
        shard = np.empty((npc, D), dtype=F32)
        shard[pc["perm"]] = real
        out[c * npc:(c + 1) * npc] = shard
    if isinstance(_run_opts, dict):
        _run_opts["_result"] = res
    return out
